# revision 5
# baseline (speedup 1.0000x reference)
"""Causal multi-head self-attention on 8 trn2 NeuronCores.

Problem: in_features [2,2048,1024], Wq/Wk/Wv/Wo [1024,1024], 16 heads,
head_dim 64, causal softmax attention, out = ctx @ Wo.

Sharding (host-side, hardcoded): core = b*4 + g for batch b in {0,1} and
head-group g in {0..3} (4 heads per group).  Each core receives
  xT   = in_features[b].T                  [1024, 2048]   (host transpose)
  wq/wk/wv = W*[:, 256g:256(g+1)]          [1024, 256]    (column shard)
  wo   = Wo[256g:256(g+1), :]              [256, 1024]    (row shard)
and returns the partial product y_partial = ctx_g @ wo_g  [2048, 1024]
as bf16.  Host sums the 4 partials per batch in fp32 (Megatron
row-parallel reduction).

On-device dataflow (per core, all SBUF operands bf16, PSUM fp32):
  qT/kT = (x @ Wq/Wk)^T  computed directly as W^T x^T  -> [256, 2048]
          stored as 2 stacked SBUF tiles [128, 2048] (head pairs).
  v     = x @ Wv in natural [S, 256] orientation, stored per k-tile with
          an appended ones column (v_aug [128, 65] per head): the ones
          column makes the ctx matmul also produce the softmax
          denominator l as output column 64.
  scoresT[k, q] = kT_tile.T @ qT_chunk  (keys on partitions).  Softmax
          without max subtraction (scores ~ N(0,1) after the 1/8 scale
          folded into the exp activation).
  p     = exp(scoresT / 8) masked multiplicatively on the diagonal band.
  ctx   = FLIPPED accumulation: out[q_tile, head] [128, 65] with the
          p tile [128 keys, 128 q] as the STATIONARY operand and
          v_aug [128 keys, 65] as the moving operand.  The cost model
          charges matmuls by moving-free-dim rows only, so this is
          65 rows/(k-tile,q-tile,head) instead of 512 rows/(k-tile,
          head) for the [65, q] layout -- 2.1x less PE time for ctx.
          Accumulators: one PSUM bank per q-subtile, 4 head slots of
          128 cols each ([128, 4, 128] fp32).
  norm  = denominator is ctx column 64; DVE reciprocal (one strided op
          for 4 heads) + per-partition tensor_scalar multiply -> bf16.
  ctxT  = PE transpose (identity trick, 128 rows/instr) back to
          [feat, q] layout required by the output projection, written
          into the spent ctx accumulator bank; Pool copies to SBUF.
  y     = sum over head-pairs of ctxT_pair.T @ wo_pair, staged bf16.
"""

import sys

if "/opt/trn_rl_repo" not in sys.path:
    sys.path.insert(0, "/opt/trn_rl_repo")

import numpy as np

import concourse.bass as bass
import concourse.mybir as mybir
import concourse.tile as tile
from concourse.bass_utils import run_bass_kernel_spmd
from concourse.vector_clock import ScopedClock

# ---------------------------------------------------------------- shapes
B = 2
S = 2048
D = 1024
H = 16
DH = 64
NCORES = 8
HLOC = 4          # heads per core
DLOC = HLOC * DH  # 256 features per core
CH = 512          # q-chunk (matmul moving-operand free dim)
NCH = S // CH     # 4
KT = 128          # k-tile (contraction tile on S)
NKT = S // KT     # 16
KPD = 8           # D // 128 k-tiles for the projections

F32 = mybir.dt.float32
BF16 = mybir.dt.bfloat16

_MAXW = 1


def _patched_drain_and_barrier(self, tick_clock, wait_clock):
    """Stock TileContext puts every outstanding sem wait on one InstDrain;
    this walrus build rejects >1 sync wait per TPB_CTRL instruction, so
    emit one drain per wait instead."""
    drain_inst = self.nc.sync.drain()
    wait_clock.add_sem_waits(
        drain_inst.ins, ScopedClock({None: tick_clock.global_clock})
    )
    si = drain_inst.ins.sync_info
    waits = list(si.on_wait) if si is not None else []
    if len(waits) > _MAXW:
        drain_inst.ins.sync_info = mybir.SyncInfo(
            on_wait=waits[:_MAXW], on_update=list(si.on_update)
        )
        for i in range(_MAXW, len(waits), _MAXW):
            d = self.nc.sync.drain()
            d.ins.sync_info = mybir.SyncInfo(
                on_wait=waits[i : i + _MAXW], on_update=[]
            )
    self.nc.all_engine_barrier()
    popped = self.nc._tile_sem_poison_stack.pop()
    assert popped is self._sem_poison
    self.nc.clear_and_free_semaphores(list(self.sems.allocated().values()))
    self.nc.all_engine_barrier()


tile.TileContext._drain_and_barrier = _patched_drain_and_barrier

_orig_commit = tile.TileContext._commit_instruction


def _patched_commit_instruction(self, inst, lazy_reg_writes=True):
    """Split instructions carrying >1 sync wait: this walrus build accepts
    at most one sync wait command per instruction, so park the excess on
    same-engine NoOps committed immediately before."""
    si = inst.sync_info
    if si is not None and len(si.on_wait) > _MAXW:
        waits = list(si.on_wait)
        extra, keep = waits[:-_MAXW], waits[-_MAXW:]
        for i in range(0, len(extra), _MAXW):
            nop = mybir.InstNoOp(
                name=self.nc.get_next_instruction_name(),
                sync_info=mybir.SyncInfo(
                    on_wait=extra[i : i + _MAXW], on_update=[]
                ),
                bass_nofuse=True,
                engine=inst.engine,
            )
            _orig_commit(self, nop, lazy_reg_writes)
        inst.sync_info = mybir.SyncInfo(
            on_wait=keep, on_update=list(si.on_update)
        )
    return _orig_commit(self, inst, lazy_reg_writes)


tile.TileContext._commit_instruction = _patched_commit_instruction


def build_nc() -> bass.Bass:
    nc = bass.Bass("TRN2", target_bir_lowering=False)

    xT = nc.dram_tensor("xT", [D, S], BF16, kind="ExternalInput")
    wq = nc.dram_tensor("wq", [D, DLOC], BF16, kind="ExternalInput")
    wk = nc.dram_tensor("wk", [D, DLOC], BF16, kind="ExternalInput")
    wv = nc.dram_tensor("wv", [D, DLOC], BF16, kind="ExternalInput")
    wo = nc.dram_tensor("wo", [DLOC, D], BF16, kind="ExternalInput")
    msk = nc.dram_tensor("msk", [KT, KT], BF16, kind="ExternalInput")
    idn = nc.dram_tensor("idn", [128, 128], BF16, kind="ExternalInput")
    y = nc.dram_tensor("y", [S, D], BF16, kind="ExternalOutput")

    Exp = mybir.ActivationFunctionType.Exp

    with nc.allow_low_precision(reason="bf16 storage for matmul operands"), \
         tile.TileContext(nc) as tc:
        with (
            tc.tile_pool(name="const", bufs=1) as const,
            tc.tile_pool(name="xin", bufs=2) as xin,
            tc.tile_pool(name="pp", bufs=6) as p_pool,
            tc.tile_pool(name="yy", bufs=3) as y_pool,
            tc.tile_pool(name="sm", bufs=4) as small,
            tc.tile_pool(name="ps", bufs=2, space="PSUM") as ps,
            tc.tile_pool(name="ctx", bufs=1, space="PSUM") as ctxpool,
        ):
            # ---------------- constants / persistent buffers
            wq_sb = [
                const.tile([128, DLOC], BF16, tag=f"wq{k}", name=f"wq{k}")
                for k in range(KPD)
            ]
            wk_sb = [
                const.tile([128, DLOC], BF16, tag=f"wk{k}", name=f"wk{k}")
                for k in range(KPD)
            ]
            wv_sb = [
                const.tile([128, DLOC], BF16, tag=f"wv{k}", name=f"wv{k}")
                for k in range(KPD)
            ]
            wo_sb = const.tile([128, 2, D], BF16, tag="wo")
            mask_sb = const.tile([128, KT], BF16, tag="mask")
            idn_sb = const.tile([128, 128], BF16, tag="idn")

            qt_sb = [
                const.tile([128, S], BF16, tag=f"qt{j}", name=f"qt{j}")
                for j in range(2)
            ]
            kt_sb = [
                const.tile([128, S], BF16, tag=f"kt{j}", name=f"kt{j}")
                for j in range(2)
            ]
            cx_sb = [
                const.tile([128, S], BF16, tag=f"cx{j}", name=f"cx{j}")
                for j in range(2)
            ]
            vaug = const.tile([128, NKT, HLOC, DH + 1], BF16, tag="vaug")
            # write bf16 1.0's bit pattern for the ones column
            nc.vector.memset(
                vaug[:, :, :, DH : DH + 1].bitcast(mybir.dt.uint16), 0x3F80
            )

            # per-q-subtile ctx accumulators: one PSUM bank each, head
            # h_idx occupies [:, h_idx, 0:65] (cols 65..127 are scratch;
            # after normalize the bank is reused as transpose output)
            ctx_ps = [
                ctxpool.tile([128, HLOC, 128], F32, tag=f"ctx{si}",
                             name=f"ctx{si}")
                for si in range(4)
            ]

            def emit_proj(c):
                cs = slice(c * CH, (c + 1) * CH)
                xt = []
                if c == 0:
                    # stream (wq_k, xt_k) pairs first so the first
                    # projection pass starts after ~2 small DMAs
                    for k in range(KPD):
                        xk = xin.tile(
                            [128, CH], BF16, tag=f"xt{k}", name=f"xt{k}",
                            bufs=1,
                        )
                        nc.sync.dma_start(
                            out=wq_sb[k], in_=wq[k * 128 : (k + 1) * 128, :]
                        )
                        nc.sync.dma_start(
                            out=xk,
                            in_=xT[k * 128 : (k + 1) * 128, cs],
                        )
                        xt.append(xk)
                    for k in range(KPD):
                        nc.sync.dma_start(
                            out=wk_sb[k], in_=wk[k * 128 : (k + 1) * 128, :]
                        )
                else:
                    xt3 = xin.tile([128, KPD, CH], BF16, tag="xtm", name="xtm")
                    nc.sync.dma_start(
                        out=xt3,
                        in_=xT.rearrange("(kt p) s -> p kt s", p=128)[:, :, cs],
                    )
                    xt = [xt3[:, k, :] for k in range(KPD)]
                # qT / kT projections (transposed layout); PSUM->SBUF
                # copies on Pool (ACT is saturated by exp, DVE by the
                # normalize/y traffic)
                for w_sb, dest in ((wq_sb, qt_sb), (wk_sb, kt_sb)):
                    for mh in range(2):
                        pt = ps.tile([128, CH], F32, tag="ps", name="pt")
                        for k in range(KPD):
                            nc.tensor.matmul(
                                pt,
                                w_sb[k][:, mh * 128 : (mh + 1) * 128],
                                xt[k],
                                start=(k == 0),
                                stop=(k == KPD - 1),
                            )
                        nc.gpsimd.tensor_copy(dest[mh][:, cs], pt)
                # v projection (natural layout, + ones col)
                if c == 0:
                    for k in range(KPD):
                        nc.sync.dma_start(
                            out=wv_sb[k], in_=wv[k * 128 : (k + 1) * 128, :]
                        )
                    nc.sync.dma_start(out=mask_sb, in_=msk[:, :])
                    nc.sync.dma_start(out=idn_sb, in_=idn[:, :])
                    nc.sync.dma_start(
                        out=wo_sb, in_=wo.rearrange("(j p) n -> p j n", p=128)
                    )
                for si in range(4):
                    st = 4 * c + si
                    pv = ps.tile([128, DLOC], F32, tag="ps", name="pv")
                    for k in range(KPD):
                        nc.tensor.matmul(
                            pv,
                            xt[k][:, si * 128 : (si + 1) * 128],
                            wv_sb[k],
                            start=(k == 0),
                            stop=(k == KPD - 1),
                        )
                    # one strided copy covers all 4 heads' [128,64] slices
                    nc.vector.tensor_copy(
                        vaug[:, st, :, 0:DH],
                        pv.rearrange("p (h d) -> p h d", h=HLOC),
                    )

            def finalize_si(c, si):
                """Normalize ctx for q-subtile si, transpose back to
                [feat, q] via the PE, park ctxT in the spent accumulator
                bank, and Pool-copy to cx_sb."""
                t = 4 * c + si
                cps = ctx_ps[si]
                rcp = small.tile([128, HLOC, 1], F32, tag="rcp", name="rcp")
                nc.vector.reciprocal(rcp, cps[:, :, DH : DH + 1])
                cxn = small.tile([128, 2, 128], BF16, tag="cxn", name="cxn")
                for h_idx in range(HLOC):
                    nc.vector.tensor_scalar_mul(
                        cxn[:, h_idx // 2, (h_idx % 2) * DH : (h_idx % 2 + 1) * DH],
                        cps[:, h_idx, 0:DH],
                        rcp[:, h_idx : h_idx + 1, :],
                    )
                # transposes: pair j's bf16 ctxT lands in the first half
                # of the spent accumulator slot j (bitcast view; heads
                # 2j -> partitions 0:64, 2j+1 -> partitions 64:128)
                for j in range(2):
                    nc.tensor.matmul(
                        cps[0:DH, j, 0:DH].bitcast(BF16),
                        cxn[:, j, 0:DH], idn_sb,
                        is_transpose=True, start=True, stop=True,
                    )
                    nc.tensor.matmul(
                        cps[DH:128, j, 0:DH].bitcast(BF16),
                        cxn[:, j, DH:128], idn_sb,
                        is_transpose=True, start=True, stop=True,
                    )
                for j in range(2):
                    nc.gpsimd.tensor_copy(
                        cx_sb[j][:, t * KT : (t + 1) * KT],
                        cps[:, j, 0:DH].bitcast(BF16),
                    )

            def kloop(c, hp):
                # scores + exp + mask + flipped ctx accumulation for a
                # head pair; per-pair p tiles use columns [0:CH] for h0,
                # [CH:] for h1.  For hp==1 the per-subtile finalize is
                # emitted as soon as that subtile's last ctx matmul is
                # drained, so the DVE/transpose chain hides under the
                # remaining k-tiles / following projections.
                nkt = 4 * (c + 1)

                def ctx_mm(k, p_sb):
                    si0 = max(0, k - 4 * c)
                    for si in range(si0, 4):
                        for h in range(2):
                            h_idx = 2 * hp + h
                            nc.tensor.matmul(
                                ctx_ps[si][:, h_idx, 0 : DH + 1],
                                p_sb[:, h * CH + si * KT : h * CH + (si + 1) * KT],
                                vaug[:, k, h_idx, :],
                                start=(k == 0),
                                stop=(k == 4 * c + si),
                            )
                    # k == 4c+si is subtile si's last k-tile: both pairs
                    # are complete once pair 1 drains it
                    if hp == 1 and k >= 4 * c:
                        finalize_si(c, k - 4 * c)

                pending = []
                for k in range(nkt):
                    # diagonal-band tiles only need columns >= w0
                    w0 = max(0, (k - 4 * c) * KT)
                    diag = k >= 4 * c
                    ksl = slice(k * KT, (k + 1) * KT)
                    qsl = slice(c * CH + w0, (c + 1) * CH)
                    sp = ps.tile([128, 2 * CH], F32, tag="ps", name="sp")
                    nc.tensor.matmul(
                        sp[:, w0:CH], kt_sb[hp][0:DH, ksl],
                        qt_sb[hp][0:DH, qsl], start=True, stop=True,
                    )
                    nc.tensor.matmul(
                        sp[:, CH + w0 :], kt_sb[hp][DH:, ksl],
                        qt_sb[hp][DH:, qsl], start=True, stop=True,
                    )
                    p_sb = p_pool.tile([128, 2 * CH], BF16, tag="p", name="p_sb")
                    # one activation covering both heads' valid columns
                    nc.scalar.activation(
                        p_sb[:, w0:], sp[:, w0:], Exp, scale=0.125
                    )
                    if diag:
                        nc.vector.tensor_mul(
                            p_sb[:, w0 : w0 + KT],
                            p_sb[:, w0 : w0 + KT], mask_sb,
                        )
                        nc.vector.tensor_mul(
                            p_sb[:, CH + w0 : CH + w0 + KT],
                            p_sb[:, CH + w0 : CH + w0 + KT], mask_sb,
                        )
                    # ctx of k-2 lands after scores of k so the PE never
                    # waits on the exp of recent tiles
                    pending.append((k, p_sb))
                    if len(pending) > 2:
                        ctx_mm(*pending.pop(0))
                for item in pending:
                    ctx_mm(*item)

            def emit_wo(c):
                # output projection for this chunk's q-tiles
                for si in range(4):
                    t = 4 * c + si
                    ysb = y_pool.tile([128, D], BF16, tag="y", name="ysb")
                    for nh in range(2):
                        yp = ps.tile([128, CH], F32, tag="ps", name="yp")
                        for j in range(2):
                            nc.tensor.matmul(
                                yp,
                                cx_sb[j][:, t * KT : (t + 1) * KT],
                                wo_sb[:, j, nh * CH : (nh + 1) * CH],
                                start=(j == 0),
                                stop=(j == 1),
                            )
                        nc.vector.tensor_copy(
                            ysb[:, nh * CH : (nh + 1) * CH], yp
                        )
                    nc.sync.dma_start(out=y[t * KT : (t + 1) * KT, :], in_=ysb)

            # warm up the PE (pstate / HAM ramp) against the first weight
            # k-slice while the remaining startup DMAs stream in
            warm = ps.tile([128, DLOC], F32, tag="ps", name="warm")
            for r in range(12):
                nc.tensor.matmul(
                    warm, wq_sb[0][:, 0:128], wq_sb[0],
                    start=(r == 0), stop=(r == 11),
                )

            # cross-chunk software pipeline: wo(c) is emitted after
            # proj(c+1) so the Pool copies feeding it are long done and
            # the PE transition kloop->proj->wo->kloop stays dense
            wo_pend = None
            for c in range(NCH):
                emit_proj(c)
                if wo_pend is not None:
                    emit_wo(wo_pend)
                    wo_pend = None
                kloop(c, 0)
                kloop(c, 1)
                wo_pend = c
            emit_wo(wo_pend)

    return nc


def _host_inputs(in_features, Wq, Wk, Wv, Wo):
    """Shard the full inputs into the 8 per-core input maps."""
    import ml_dtypes

    bf16 = ml_dtypes.bfloat16
    x = np.asarray(in_features, dtype=np.float32)
    Wq = np.asarray(Wq, dtype=np.float32).astype(bf16)
    Wk = np.asarray(Wk, dtype=np.float32).astype(bf16)
    Wv = np.asarray(Wv, dtype=np.float32).astype(bf16)
    Wo = np.asarray(Wo, dtype=np.float32).astype(bf16)

    # triangular causal mask for the exact-diagonal 128x128 block
    ki = np.arange(KT)[:, None]
    qj = np.arange(KT)[None, :]
    msk = (ki <= qj).astype(bf16)
    idn = np.eye(128, dtype=bf16)

    xTs = [np.ascontiguousarray(x[b].T).astype(bf16) for b in range(B)]
    in_maps = []
    for core in range(NCORES):
        b, g = divmod(core, 4)
        colsl = slice(g * DLOC, (g + 1) * DLOC)
        in_maps.append(
            {
                "xT": xTs[b],
                "wq": np.ascontiguousarray(Wq[:, colsl]),
                "wk": np.ascontiguousarray(Wk[:, colsl]),
                "wv": np.ascontiguousarray(Wv[:, colsl]),
                "wo": np.ascontiguousarray(Wo[colsl, :]),
                "msk": msk,
                "idn": idn,
            }
        )
    return in_maps


_NC_CACHE = None


def _get_nc():
    global _NC_CACHE
    if _NC_CACHE is None:
        _NC_CACHE = build_nc()
    return _NC_CACHE


def kernel(in_features, Wq, Wk, Wv, Wo):
    in_maps = _host_inputs(in_features, Wq, Wk, Wv, Wo)
    nc = _get_nc()
    res = run_bass_kernel_spmd(nc, in_maps, core_ids=list(range(NCORES)))
    parts = [res.results[core]["y"] for core in range(NCORES)]
    out = np.empty((B, S, D), dtype=np.float32)
    for b in range(B):
        acc = parts[4 * b].astype(np.float32)
        for g in range(1, 4):
            acc = acc + parts[4 * b + g].astype(np.float32)
        out[b] = acc
    return out


# revision 10
# speedup vs baseline: 1.0350x; 1.0350x over previous
"""Causal multi-head self-attention on 8 trn2 NeuronCores.

Problem: in_features [2,2048,1024], Wq/Wk/Wv/Wo [1024,1024], 16 heads,
head_dim 64, causal softmax attention, out = ctx @ Wo.

Sharding (host-side, hardcoded): core = b*4 + g for batch b in {0,1} and
head-group g in {0..3} (4 heads per group).  Each core receives
  xT   = in_features[b].T                  [1024, 2048]   (host transpose)
  wq/wk/wv = W*[:, 256g:256(g+1)]          [1024, 256]    (column shard)
  wo   = Wo[256g:256(g+1), :]              [256, 1024]    (row shard)
and returns the partial product y_partial = ctx_g @ wo_g  [2048, 1024]
as bf16.  Host sums the 4 partials per batch in fp32 (Megatron
row-parallel reduction).

On-device dataflow (per core, all SBUF operands bf16, PSUM fp32):
  qT/kT = (x @ Wq/Wk)^T  computed directly as W^T x^T  -> [256, 2048]
          stored as 2 stacked SBUF tiles [128, 2048] (head pairs).
  v     = x @ Wv in natural [S, 256] orientation, stored per k-tile with
          an appended ones column (v_aug [128, 65] per head): the ones
          column makes the ctx matmul also produce the softmax
          denominator l as output column 64.
  scoresT[k, q] = kT_tile.T @ qT_chunk  (keys on partitions).  Softmax
          without max subtraction (scores ~ N(0,1) after the 1/8 scale
          folded into the exp activation).
  p     = exp(scoresT / 8) masked multiplicatively on the diagonal band.
  ctx   = FLIPPED accumulation: out[q_tile, head] [128, 65] with the
          p tile [128 keys, 128 q] as the STATIONARY operand and
          v_aug [128 keys, 65] as the moving operand.  The cost model
          charges matmuls by moving-free-dim rows only, so this is
          65 rows/(k-tile,q-tile,head) instead of 512 rows/(k-tile,
          head) for the [65, q] layout -- 2.1x less PE time for ctx.
          Accumulators: one PSUM bank per q-subtile, 4 head slots of
          128 cols each ([128, 4, 128] fp32).
  norm  = denominator is ctx column 64; DVE reciprocal (one strided op
          for 4 heads) + per-partition tensor_scalar multiply -> bf16.
  ctxT  = PE transpose (identity trick, 128 rows/instr) back to
          [feat, q] layout required by the output projection, written
          into the spent ctx accumulator bank; Pool copies to SBUF.
  y     = sum over head-pairs of ctxT_pair.T @ wo_pair, staged bf16.
"""

import sys

if "/opt/trn_rl_repo" not in sys.path:
    sys.path.insert(0, "/opt/trn_rl_repo")

import numpy as np

import concourse.bass as bass
import concourse.mybir as mybir
import concourse.tile as tile
from concourse.bass_utils import run_bass_kernel_spmd
from concourse.vector_clock import ScopedClock

# ---------------------------------------------------------------- shapes
B = 2
S = 2048
D = 1024
H = 16
DH = 64
NCORES = 8
HLOC = 4          # heads per core
DLOC = HLOC * DH  # 256 features per core
CH = 512          # q-chunk (matmul moving-operand free dim)
NCH = S // CH     # 4
KT = 128          # k-tile (contraction tile on S)
NKT = S // KT     # 16
KPD = 8           # D // 128 k-tiles for the projections

F32 = mybir.dt.float32
BF16 = mybir.dt.bfloat16

_MAXW = 1


def _patched_drain_and_barrier(self, tick_clock, wait_clock):
    """Stock TileContext puts every outstanding sem wait on one InstDrain;
    this walrus build rejects >1 sync wait per TPB_CTRL instruction, so
    emit one drain per wait instead."""
    drain_inst = self.nc.sync.drain()
    wait_clock.add_sem_waits(
        drain_inst.ins, ScopedClock({None: tick_clock.global_clock})
    )
    si = drain_inst.ins.sync_info
    waits = list(si.on_wait) if si is not None else []
    if len(waits) > _MAXW:
        drain_inst.ins.sync_info = mybir.SyncInfo(
            on_wait=waits[:_MAXW], on_update=list(si.on_update)
        )
        for i in range(_MAXW, len(waits), _MAXW):
            d = self.nc.sync.drain()
            d.ins.sync_info = mybir.SyncInfo(
                on_wait=waits[i : i + _MAXW], on_update=[]
            )
    self.nc.all_engine_barrier()
    popped = self.nc._tile_sem_poison_stack.pop()
    assert popped is self._sem_poison
    self.nc.clear_and_free_semaphores(list(self.sems.allocated().values()))
    self.nc.all_engine_barrier()


tile.TileContext._drain_and_barrier = _patched_drain_and_barrier

_orig_commit = tile.TileContext._commit_instruction


def _patched_commit_instruction(self, inst, lazy_reg_writes=True):
    """Split instructions carrying >1 sync wait: this walrus build accepts
    at most one sync wait command per instruction, so park the excess on
    same-engine NoOps committed immediately before."""
    si = inst.sync_info
    if si is not None and len(si.on_wait) > _MAXW:
        waits = list(si.on_wait)
        extra, keep = waits[:-_MAXW], waits[-_MAXW:]
        for i in range(0, len(extra), _MAXW):
            nop = mybir.InstNoOp(
                name=self.nc.get_next_instruction_name(),
                sync_info=mybir.SyncInfo(
                    on_wait=extra[i : i + _MAXW], on_update=[]
                ),
                bass_nofuse=True,
                engine=inst.engine,
            )
            _orig_commit(self, nop, lazy_reg_writes)
        inst.sync_info = mybir.SyncInfo(
            on_wait=keep, on_update=list(si.on_update)
        )
    return _orig_commit(self, inst, lazy_reg_writes)


tile.TileContext._commit_instruction = _patched_commit_instruction


def build_nc() -> bass.Bass:
    nc = bass.Bass("TRN2", target_bir_lowering=False)

    xT = nc.dram_tensor("xT", [D, S], BF16, kind="ExternalInput")
    wq = nc.dram_tensor("wq", [D, DLOC], BF16, kind="ExternalInput")
    wk = nc.dram_tensor("wk", [D, DLOC], BF16, kind="ExternalInput")
    wv = nc.dram_tensor("wv", [D, DLOC], BF16, kind="ExternalInput")
    wo = nc.dram_tensor("wo", [DLOC, D], BF16, kind="ExternalInput")
    msk = nc.dram_tensor("msk", [KT, KT], BF16, kind="ExternalInput")
    idn = nc.dram_tensor("idn", [128, 128], BF16, kind="ExternalInput")
    y = nc.dram_tensor("y", [S, D], BF16, kind="ExternalOutput")

    Exp = mybir.ActivationFunctionType.Exp

    with nc.allow_low_precision(reason="bf16 storage for matmul operands"), \
         tile.TileContext(nc) as tc:
        with (
            tc.tile_pool(name="const", bufs=1) as const,
            tc.tile_pool(name="xin", bufs=2) as xin,
            tc.tile_pool(name="pp", bufs=6) as p_pool,
            tc.tile_pool(name="yy", bufs=3) as y_pool,
            tc.tile_pool(name="sm", bufs=4) as small,
            tc.tile_pool(name="ps", bufs=2, space="PSUM") as ps,
            tc.tile_pool(name="ctx", bufs=1, space="PSUM") as ctxpool,
        ):
            # ---------------- constants / persistent buffers
            # batched weight tiles: one DMA each (HWDGE desc-gen costs
            # 625ns per dma_start, so 8 small loads serialize the
            # prologue), k-slice k at [:, k, :]
            wq_sb = const.tile([128, KPD, DLOC], BF16, tag="wq")
            wk_sb = const.tile([128, KPD, DLOC], BF16, tag="wk")
            wv_sb = const.tile([128, KPD, DLOC], BF16, tag="wv")
            wo_sb = const.tile([128, 2, D], BF16, tag="wo")
            mask_sb = const.tile([128, KT], BF16, tag="mask")
            idn_sb = const.tile([128, 128], BF16, tag="idn")

            qt_sb = [
                const.tile([128, S], BF16, tag=f"qt{j}", name=f"qt{j}")
                for j in range(2)
            ]
            kt_sb = [
                const.tile([128, S], BF16, tag=f"kt{j}", name=f"kt{j}")
                for j in range(2)
            ]
            cx_sb = [
                const.tile([128, S], BF16, tag=f"cx{j}", name=f"cx{j}")
                for j in range(2)
            ]
            vaug = const.tile([128, NKT, HLOC, DH + 1], BF16, tag="vaug")
            # write bf16 1.0's bit pattern for the ones column
            nc.vector.memset(
                vaug[:, :, :, DH : DH + 1].bitcast(mybir.dt.uint16), 0x3F80
            )

            # per-q-subtile ctx accumulators: one PSUM bank each, head
            # h_idx occupies [:, h_idx, 0:65] (cols 65..127 are scratch;
            # after normalize the bank is reused as transpose output)
            ctx_ps = [
                ctxpool.tile([128, HLOC, 128], F32, tag=f"ctx{si}",
                             name=f"ctx{si}")
                for si in range(4)
            ]

            def proj_dma(c):
                """Issue chunk c's x DMA; returns the SBUF tile."""
                cs = slice(c * CH, (c + 1) * CH)
                xt3 = xin.tile([128, KPD, CH], BF16, tag="xtm", name="xtm")
                nc.sync.dma_start(
                    out=xt3,
                    in_=xT.rearrange("(kt p) s -> p kt s", p=128)[:, :, cs],
                )
                return xt3

            def make_proj_fillers(c, xt3):
                """Chunk c's projection matmuls as 8 PE work groups, to be
                interleaved between k-tiles of the previous chunk's
                attention loop (the ACT exp stream paces that loop, so
                the PE has slack there).  PSUM->SBUF copies on Pool
                (qt/kt) and DVE (v)."""
                cs = slice(c * CH, (c + 1) * CH)
                xt = [xt3[:, k, :] for k in range(KPD)]
                fillers = []

                def qk_group(w_sb, dest, mh):
                    pt = ps.tile([128, CH], F32, tag="ps", name="pt")
                    for k in range(KPD):
                        nc.tensor.matmul(
                            pt,
                            w_sb[:, k, mh * 128 : (mh + 1) * 128],
                            xt[k],
                            start=(k == 0),
                            stop=(k == KPD - 1),
                        )
                    nc.gpsimd.tensor_copy(dest[mh][:, cs], pt)

                def v_group(si):
                    st = 4 * c + si
                    pv = ps.tile([128, DLOC], F32, tag="ps", name="pv")
                    for k in range(KPD):
                        nc.tensor.matmul(
                            pv,
                            xt[k][:, si * 128 : (si + 1) * 128],
                            wv_sb[:, k, :],
                            start=(k == 0),
                            stop=(k == KPD - 1),
                        )
                    # one strided copy covers all 4 heads' [128,64] slices
                    nc.vector.tensor_copy(
                        vaug[:, st, :, 0:DH],
                        pv.rearrange("p (h d) -> p h d", h=HLOC),
                    )

                import functools
                for w_sb, dest in ((wq_sb, qt_sb), (wk_sb, kt_sb)):
                    for mh in range(2):
                        fillers.append(
                            functools.partial(qk_group, w_sb, dest, mh)
                        )
                for si in range(4):
                    fillers.append(functools.partial(v_group, si))
                return fillers

            def finalize_si(c, si):
                """Normalize ctx for q-subtile si, transpose back to
                [feat, q] via the PE, park ctxT in the spent accumulator
                bank, and Pool-copy to cx_sb."""
                t = 4 * c + si
                cps = ctx_ps[si]
                rcp = small.tile([128, HLOC, 1], F32, tag="rcp", name="rcp")
                nc.vector.reciprocal(rcp, cps[:, :, DH : DH + 1])
                cxn = small.tile([128, 2, 128], BF16, tag="cxn", name="cxn")
                for h_idx in range(HLOC):
                    nc.vector.tensor_scalar_mul(
                        cxn[:, h_idx // 2, (h_idx % 2) * DH : (h_idx % 2 + 1) * DH],
                        cps[:, h_idx, 0:DH],
                        rcp[:, h_idx : h_idx + 1, :],
                    )
                # one [128,128] transpose per pair: cxn[:, j, :] is
                # [q, 2 heads x 64 feat], its transpose is exactly the
                # stacked [feat, q] block the output projection wants;
                # lands bf16 in the spent accumulator slot j (bitcast)
                for j in range(2):
                    nc.tensor.matmul(
                        cps[:, j, 0:DH].bitcast(BF16),
                        cxn[:, j, :], idn_sb,
                        is_transpose=True, start=True, stop=True,
                    )
                for j in range(2):
                    nc.gpsimd.tensor_copy(
                        cx_sb[j][:, t * KT : (t + 1) * KT],
                        cps[:, j, 0:DH].bitcast(BF16),
                    )

            def kloop(c, hp, fillers, skip=0):
                # scores + exp + mask + flipped ctx accumulation for a
                # head pair; per-pair p tiles use columns [0:CH] for h0,
                # [CH:] for h1.  For hp==1 the per-subtile finalize is
                # emitted as soon as that subtile's last ctx matmul is
                # drained, so the DVE/transpose chain hides under the
                # remaining k-tiles / following projections.  After each
                # k-tile one pending PE filler group (next chunk's
                # projections / previous chunk's output projection) is
                # emitted: the exp stream paces this loop on ACT, so the
                # PE has ~1/3 idle here without them.
                nkt = 4 * (c + 1)

                def ctx_mm(k, p_sb):
                    si0 = max(0, k - 4 * c)
                    for si in range(si0, 4):
                        for h in range(2):
                            h_idx = 2 * hp + h
                            nc.tensor.matmul(
                                ctx_ps[si][:, h_idx, 0 : DH + 1],
                                p_sb[:, h * CH + si * KT : h * CH + (si + 1) * KT],
                                vaug[:, k, h_idx, :],
                                start=(k == 0),
                                stop=(k == 4 * c + si),
                            )
                    # k == 4c+si is subtile si's last k-tile: both pairs
                    # are complete once pair 1 drains it
                    if hp == 1 and k >= 4 * c:
                        finalize_si(c, k - 4 * c)

                pending = []
                ev = 0
                for k in range(nkt):
                    # diagonal-band tiles only need columns >= w0
                    w0 = max(0, (k - 4 * c) * KT)
                    diag = k >= 4 * c
                    ksl = slice(k * KT, (k + 1) * KT)
                    qsl = slice(c * CH + w0, (c + 1) * CH)
                    sp = ps.tile([128, 2 * CH], F32, tag="ps", name="sp")
                    nc.tensor.matmul(
                        sp[:, w0:CH], kt_sb[hp][0:DH, ksl],
                        qt_sb[hp][0:DH, qsl], start=True, stop=True,
                    )
                    nc.tensor.matmul(
                        sp[:, CH + w0 :], kt_sb[hp][DH:, ksl],
                        qt_sb[hp][DH:, qsl], start=True, stop=True,
                    )
                    p_sb = p_pool.tile([128, 2 * CH], BF16, tag="p", name="p_sb")
                    # one activation covering both heads' valid columns
                    nc.scalar.activation(
                        p_sb[:, w0:], sp[:, w0:], Exp, scale=0.125
                    )
                    if diag:
                        nc.vector.tensor_mul(
                            p_sb[:, w0 : w0 + KT],
                            p_sb[:, w0 : w0 + KT], mask_sb,
                        )
                        nc.vector.tensor_mul(
                            p_sb[:, CH + w0 : CH + w0 + KT],
                            p_sb[:, CH + w0 : CH + w0 + KT], mask_sb,
                        )
                    # ctx of k-2 lands after scores of k so the PE never
                    # waits on the exp of recent tiles
                    pending.append((k, p_sb))
                    if len(pending) > 2:
                        ctx_mm(*pending.pop(0))
                    ev += 1
                    if fillers and ev > skip:
                        fillers.popleft()()
                for item in pending:
                    ctx_mm(*item)
                    if fillers:
                        fillers.popleft()()

            def make_wo_fillers(c):
                """Output projection for chunk c's q-tiles as 4 PE work
                groups (one per q-tile), interleaved into the next
                chunk's attention loop."""
                def wo_group(si):
                    t = 4 * c + si
                    ysb = y_pool.tile([128, D], BF16, tag="y", name="ysb")
                    for nh in range(2):
                        yp = ps.tile([128, CH], F32, tag="ps", name="yp")
                        for j in range(2):
                            nc.tensor.matmul(
                                yp,
                                cx_sb[j][:, t * KT : (t + 1) * KT],
                                wo_sb[:, j, nh * CH : (nh + 1) * CH],
                                start=(j == 0),
                                stop=(j == 1),
                            )
                        nc.vector.tensor_copy(
                            ysb[:, nh * CH : (nh + 1) * CH], yp
                        )
                    nc.sync.dma_start(out=y[t * KT : (t + 1) * KT, :], in_=ysb)

                import functools
                return [functools.partial(wo_group, si) for si in range(4)]

            # ---------------- startup: batched DMAs, then PE warmup
            nc.sync.dma_start(
                out=wq_sb, in_=wq.rearrange("(kt p) n -> p kt n", p=128)
            )
            xt3_0 = proj_dma(0)
            nc.sync.dma_start(
                out=wk_sb, in_=wk.rearrange("(kt p) n -> p kt n", p=128)
            )
            nc.sync.dma_start(
                out=wv_sb, in_=wv.rearrange("(kt p) n -> p kt n", p=128)
            )
            nc.sync.dma_start(out=mask_sb, in_=msk[:, :])
            nc.sync.dma_start(out=idn_sb, in_=idn[:, :])
            nc.sync.dma_start(
                out=wo_sb, in_=wo.rearrange("(j p) n -> p j n", p=128)
            )

            # warm up the PE (pstate / HAM ramp) against the first weight
            # k-slice while the remaining startup DMAs stream in
            warm = ps.tile([128, DLOC], F32, tag="ps", name="warm")
            for r in range(12):
                nc.tensor.matmul(
                    warm, wq_sb[:, 0, 0:128], wq_sb[:, 0, :],
                    start=(r == 0), stop=(r == 11),
                )

            # chunk 0's projections run directly (nothing to overlap yet)
            from collections import deque
            fillers = deque()
            for g in make_proj_fillers(0, xt3_0):
                g()

            # cross-chunk software pipeline: chunk c+1's projections and
            # chunk c-1's output projection are fillers inside chunk c's
            # ACT-paced attention loops
            for c in range(NCH):
                if c + 1 < NCH:
                    xt3 = proj_dma(c + 1)
                    fillers.extend(make_proj_fillers(c + 1, xt3))
                kloop(c, 0, fillers, skip=2 if c == 0 else 0)
                kloop(c, 1, fillers)
                while fillers:
                    fillers.popleft()()
                fillers.extend(make_wo_fillers(c))
            while fillers:
                fillers.popleft()()

    return nc


def _host_inputs(in_features, Wq, Wk, Wv, Wo):
    """Shard the full inputs into the 8 per-core input maps."""
    import ml_dtypes

    bf16 = ml_dtypes.bfloat16
    x = np.asarray(in_features, dtype=np.float32)
    Wq = np.asarray(Wq, dtype=np.float32).astype(bf16)
    Wk = np.asarray(Wk, dtype=np.float32).astype(bf16)
    Wv = np.asarray(Wv, dtype=np.float32).astype(bf16)
    Wo = np.asarray(Wo, dtype=np.float32).astype(bf16)

    # triangular causal mask for the exact-diagonal 128x128 block
    ki = np.arange(KT)[:, None]
    qj = np.arange(KT)[None, :]
    msk = (ki <= qj).astype(bf16)
    idn = np.eye(128, dtype=bf16)

    xTs = [np.ascontiguousarray(x[b].T).astype(bf16) for b in range(B)]
    in_maps = []
    for core in range(NCORES):
        b, g = divmod(core, 4)
        colsl = slice(g * DLOC, (g + 1) * DLOC)
        in_maps.append(
            {
                "xT": xTs[b],
                "wq": np.ascontiguousarray(Wq[:, colsl]),
                "wk": np.ascontiguousarray(Wk[:, colsl]),
                "wv": np.ascontiguousarray(Wv[:, colsl]),
                "wo": np.ascontiguousarray(Wo[colsl, :]),
                "msk": msk,
                "idn": idn,
            }
        )
    return in_maps


_NC_CACHE = None


def _get_nc():
    global _NC_CACHE
    if _NC_CACHE is None:
        _NC_CACHE = build_nc()
    return _NC_CACHE


def kernel(in_features, Wq, Wk, Wv, Wo):
    in_maps = _host_inputs(in_features, Wq, Wk, Wv, Wo)
    nc = _get_nc()
    res = run_bass_kernel_spmd(nc, in_maps, core_ids=list(range(NCORES)))
    parts = [res.results[core]["y"] for core in range(NCORES)]
    out = np.empty((B, S, D), dtype=np.float32)
    for b in range(B):
        acc = parts[4 * b].astype(np.float32)
        for g in range(1, 4):
            acc = acc + parts[4 * b + g].astype(np.float32)
        out[b] = acc
    return out
